# revision 41
# baseline (speedup 1.0000x reference)
"""Trainium2 Bass kernel for ExpertBranch: fp8-blockwise-fakequant FFN.

  h   = gelu_tanh(fq8(x) @ fq8_rows(kernel1) + bias1)
  out = fq8(h) @ fq8_rows(kernel2) + bias2

Sharding: data-parallel over the 8192 flattened rows of x — each of the 8
NeuronCores computes a 1024-row slice with replicated weights. No collectives.

Device pipeline per core (M=1024 rows), single fused region:
  A: x blockwise-fp8 fake-quant (halved-scale TRN-e4m3 trick, exact reference
     grid) -> fp16 dequant -> XBAR DMA-transpose into resident xT (SBUF).
  B: GEMM1 (fp16 operands, fp32 PSUM) + bias1 + exact tanh-gelu chain spread
     across DVE/Act/Pool + h fake-quant -> fp16 dequant -> XBAR DMA-transpose
     into resident hT (SBUF, no DRAM round-trip).
  C: GEMM2 (fp16 x fp16) streaming w2q once + bias2 -> out.

Weights are fake-quantized on the host (numpy, bitwise-exact OCP e4m3fn
semantics) and stored fp16 — weight quantization is static preprocessing; all
activation work (x-quant, GEMMs, gelu, h-quant) runs on device.  fp16 (not
bf16) keeps operand rounding at 2^-11, which matters because GEMM1 noise is
amplified by h-quant grid-boundary flips.
"""

import contextlib
import sys

import numpy as np

sys.path.insert(0, "/opt/trn_rl_repo")

import ml_dtypes  # noqa: E402

import concourse.bacc as bacc  # noqa: E402
import concourse.bass as bass  # noqa: E402
import concourse.mybir as mybir  # noqa: E402
import concourse.tile as tile  # noqa: E402
from concourse.bass_utils import run_bass_kernel_spmd  # noqa: E402

F32 = mybir.dt.float32
F16 = mybir.dt.float16
FP8 = mybir.dt.float8e4

P = 128          # partitions
NCORES = 8
D_MODEL = 2048
EXPERT = 8192
ROWS = 4 * 2048  # flattened x rows
MC = ROWS // NCORES   # rows per core = 1024
MT = MC // P          # m-tiles per core = 8
KB1 = D_MODEL // P    # k-blocks GEMM1 = 16
J1 = 256              # GEMM1 n-tile
NT1 = EXPERT // J1    # n-tiles GEMM1 = 32
NBJ = J1 // P         # fp8 blocks per GEMM1 n-tile = 2
KB2 = EXPERT // P     # k-blocks GEMM2 = 64
KC = 16               # k-blocks per w2 stream chunk
NKC = KB2 // KC       # chunks = 8
JT = EXPERT // 512    # j-tiles GEMM2 = 16
J2 = 512

C1 = float(np.float32(np.sqrt(2.0 / np.pi)))
GA = float(np.float32(0.044715))
C224INV = float(np.float32(1.0 / 224.0))
C448INV = float(np.float32(1.0 / 448.0))
EPS = 1e-12
HD = D_MODEL // 4     # phase-A quarter-tile width = 512
HKB = KB1 // 4        # k-blocks per quarter-tile = 4


def _build():
    nc = bacc.Bacc("TRN2", target_bir_lowering=False, debug=False)

    # Packed inputs (host-prepared layouts; see kernel() below).
    x_in = nc.dram_tensor("xp", [P, MT, D_MODEL], F32, kind="ExternalInput")
    w1_in = nc.dram_tensor("w1p", [P, KB1, EXPERT], F16, kind="ExternalInput")
    b1_in = nc.dram_tensor("b1", [EXPERT], F32, kind="ExternalInput")
    w2_in = nc.dram_tensor("w2p", [P, KB2, EXPERT], F16, kind="ExternalInput")
    b2_in = nc.dram_tensor("b2", [EXPERT], F32, kind="ExternalInput")
    out = nc.dram_tensor("out", [MC, EXPERT], F32, kind="ExternalOutput")

    with tile.TileContext(nc) as tc, contextlib.ExitStack() as top:
        resid = top.enter_context(tc.tile_pool(name="resid", bufs=1))
        # xT[p, mi, kb, m'] = xq_mi[m', kb*128+p]   (32 KiB/part)
        xT = resid.tile([P, MT, KB1, P], F16)
        # hT[p, mi, kb, m'] = hq_mi[m', kb*128+p]   (128 KiB/part)
        hT = resid.tile([P, MT, KB2, P], F16)

        # ---- Phases A+B share one pool scope so SBUF reuse deps don't
        # serialize the x pipeline against GEMM1.
        with contextlib.ExitStack() as ctx:
            # Phase B pools
            w1p = ctx.enter_context(tc.tile_pool(name="w1p", bufs=2))
            b1p = ctx.enter_context(tc.tile_pool(name="b1p", bufs=2))
            zp = ctx.enter_context(tc.tile_pool(name="zp", bufs=5))
            hp2 = ctx.enter_context(tc.tile_pool(name="hp2", bufs=5))
            scb = ctx.enter_context(tc.tile_pool(name="scb", bufs=6))
            q8b = ctx.enter_context(tc.tile_pool(name="q8b", bufs=2))
            hqp = ctx.enter_context(tc.tile_pool(name="hqp", bufs=5))
            pp = ctx.enter_context(tc.tile_pool(name="pp", bufs=8, space="PSUM"))

            # head: first two n-tiles interleaved mi-major so early GEMM1
            # work unlocks per x-unit during phase A; tail: ni-major.
            jobs = [(ni, mi) for mi in range(MT) for ni in (0, 1)]
            jobs += [(ni, mi) for ni in range(2, NT1) for mi in range(MT)]
            NJ = len(jobs)
            st = {}
            w1ts = {}
            b1ts = {}

            def load_ni(ni):
                # chunked so no single DMA holds the DMA engines long enough
                # to stall a transpose dispatch behind it
                w1t = w1p.tile([P, KB1, J1], F16)
                for c in range(4):
                    nc.sync.dma_start(
                        out=w1t[:, 4 * c:4 * (c + 1), :],
                        in_=w1_in[:, 4 * c:4 * (c + 1), J1 * ni:J1 * (ni + 1)])
                b1t = b1p.tile([P, J1], F32)
                nc.sync.dma_start(
                    out=b1t[:], in_=bass.AP(b1_in, J1 * ni, [[0, P], [1, J1]]))
                w1ts[ni] = w1t
                b1ts[ni] = b1t


            # ---------------- Phase A: x quant -> fp16 -> transpose --------
            # A pools live in their own scope, closed right after emission so
            # phase C's w2 stream buffers can reuse these bytes (their DMAs
            # then depend only on long-finished A ops -> prefetch during B).
            actx = contextlib.ExitStack()
            xa = actx.enter_context(tc.tile_pool(name="xa", bufs=3))
            sca = actx.enter_context(tc.tile_pool(name="sca", bufs=3))
            q8a = actx.enter_context(tc.tile_pool(name="q8a", bufs=2))
            xqa = actx.enter_context(tc.tile_pool(name="xqa", bufs=2))
            # Skewed stages (load / amax+quant / dequant / transpose) so the
            # cross-engine ring pipelines instead of serializing per unit.
            # No PE instructions here: GEMM1 below starts as soon as the
            # first xT slices land.
            NA = MT * 4
            ast = {}

            def a_load(k):
                mi, h = divmod(k, 4)
                xt = xa.tile([P, HD], F32)
                nc.sync.dma_start(
                    out=xt[:], in_=x_in[:, mi, HD * h:HD * (h + 1)])
                if k == 0:
                    load_ni(0)
                    load_ni(1)
                ast[k] = {"xt": xt}

            def a_scales(k):
                s = ast[k]
                xv3 = s["xt"][:].rearrange("p (kb b) -> p kb b", b=P)
                amax = sca.tile([P, HKB], F32, tag="amax")
                nc.vector.tensor_reduce(
                    amax[:], xv3, axis=mybir.AxisListType.X,
                    op=mybir.AluOpType.max, apply_absolute_value=True)
                nc.vector.tensor_scalar_max(amax[:], amax[:], EPS)
                rcp = sca.tile([P, HKB], F32, tag="rcp")
                nc.vector.reciprocal(rcp[:], amax[:])
                inv2 = sca.tile([P, HKB], F32, tag="inv2")
                nc.vector.tensor_scalar_mul(inv2[:], rcp[:], 224.0)
                s2 = sca.tile([P, HKB], F32, tag="s2")
                nc.vector.tensor_scalar_mul(s2[:], amax[:], C224INV)
                s["inv2"] = inv2
                s["s2"] = s2

            def a_quant(k):
                s = ast[k]
                q8 = q8a.tile([P, HD], FP8)
                for kb in range(HKB):
                    sl = slice(P * kb, P * (kb + 1))
                    # fp8 code: RNE(fl32(x * (224/amax))), split Act/DVE
                    if kb % 4 < 2:
                        nc.scalar.activation(
                            q8[:, sl], s["xt"][:, sl],
                            mybir.ActivationFunctionType.Copy,
                            scale=s["inv2"][:, kb:kb + 1])
                    else:
                        nc.vector.tensor_scalar(
                            q8[:, sl], s["xt"][:, sl], s["inv2"][:, kb:kb + 1],
                            None, op0=mybir.AluOpType.mult)
                s["q8"] = q8

            def a_dequant(k):
                s = ast[k]
                xq = xqa.tile([P, HD], F16)
                for kb in range(HKB):
                    sl = slice(P * kb, P * (kb + 1))
                    # dequant: fp16(code * fl(amax/224)), split Pool/Act
                    if kb % 4 < 3:
                        nc.gpsimd.tensor_scalar(
                            xq[:, sl], s["q8"][:, sl], s["s2"][:, kb:kb + 1],
                            None, op0=mybir.AluOpType.mult)
                    else:
                        nc.scalar.activation(
                            xq[:, sl], s["q8"][:, sl],
                            mybir.ActivationFunctionType.Copy,
                            scale=s["s2"][:, kb:kb + 1])
                s["xq"] = xq

            def a_transpose(k):
                mi, h = divmod(k, 4)
                s = ast.pop(k)
                nc.sync.dma_start_transpose(
                    out=xT[:, mi, HKB * h:HKB * (h + 1), :], in_=s["xq"][:])

            for k in range(NA + 4):
                if k < NA:
                    a_load(k)
                if 0 <= k - 1 < NA:
                    a_scales(k - 1)
                if 0 <= k - 2 < NA:
                    a_quant(k - 2)
                if 0 <= k - 3 < NA:
                    a_dequant(k - 3)
                if 0 <= k - 4 < NA:
                    a_transpose(k - 4)
            actx.close()

            # ------- Phase B: GEMM1 + bias + gelu + h-quant + transpose ----
            # Software-pipelined emission: stage s of tile j is emitted next
            # to stage s-1 of tile j+1 so no in-order engine queue ever
            # blocks on a same-tile cross-engine dependency.

            def emit_s0(j):
                ni, mi = jobs[j]
                tgt = 2 if j == 13 else (ni + 1 if j >= 16 and mi == 0 else -1)
                if 0 <= tgt < NT1 and tgt not in w1ts:
                    load_ni(tgt)
                ps = pp.tile([P, J1], F32)
                for kb in range(KB1):
                    nc.tensor.matmul(
                        ps[:], xT[:, mi, kb, :], w1ts[ni][:, kb, :],
                        start=(kb == 0), stop=(kb == KB1 - 1))
                z = zp.tile([P, J1], F32, tag="z")
                nc.vector.tensor_tensor(
                    z[:], ps[:], b1ts[ni][:], op=mybir.AluOpType.add)
                st[j] = {"z": z}

            def emit_s2(j):
                s = st[j]
                # h = gelu_tanh(z) via the hardware act table (matches the
                # jax tanh-approx gelu formula)
                h = hp2.tile([P, J1], F32, tag="h")
                nc.scalar.activation(
                    h[:], s["z"][:],
                    mybir.ActivationFunctionType.Gelu_apprx_tanh)
                s["h"] = h

            def emit_s3(j):
                s = st[j]
                amaxh = scb.tile([P, NBJ], F32, tag="amaxh")
                nc.vector.tensor_reduce(
                    amaxh[:], s["h"][:].rearrange("p (nb b) -> p nb b", b=P),
                    axis=mybir.AxisListType.X,
                    op=mybir.AluOpType.max, apply_absolute_value=True)
                nc.vector.tensor_scalar_max(amaxh[:], amaxh[:], EPS)
                rch = scb.tile([P, NBJ], F32, tag="rch")
                nc.vector.reciprocal(rch[:], amaxh[:])
                inv2h = scb.tile([P, NBJ], F32, tag="inv2h")
                nc.vector.tensor_scalar_mul(inv2h[:], rch[:], 224.0)
                s2h = scb.tile([P, NBJ], F32, tag="s2h")
                nc.vector.tensor_scalar_mul(s2h[:], amaxh[:], C224INV)
                s["inv2h"] = inv2h
                s["s2h"] = s2h

            def emit_s4(j):
                s = st[j]
                h8 = q8b.tile([P, J1], FP8, tag="h8")
                hq = hqp.tile([P, J1], F16, tag="hq")
                for b in range(NBJ):
                    sl = slice(P * b, P * (b + 1))
                    nc.scalar.activation(
                        h8[:, sl], s["h"][:, sl],
                        mybir.ActivationFunctionType.Copy,
                        scale=s["inv2h"][:, b:b + 1])
                    nc.vector.tensor_scalar(
                        hq[:, sl], h8[:, sl], s["s2h"][:, b:b + 1], None,
                        op0=mybir.AluOpType.mult)
                s["hq"] = hq

            def emit_s5(j):
                ni, mi = jobs[j]
                s = st.pop(j)
                eng = nc.scalar if j >= NJ - 8 else nc.sync
                eng.dma_start_transpose(
                    out=hT[:, mi, NBJ * ni:NBJ * (ni + 1), :], in_=s["hq"][:])

            for j in range(NJ + 8):
                if j < NJ:
                    emit_s0(j)
                if 0 <= j - 2 < NJ:
                    emit_s2(j - 2)
                if 0 <= j - 4 < NJ:
                    emit_s3(j - 4)
                if 0 <= j - 6 < NJ:
                    emit_s4(j - 6)
                if 0 <= j - 8 < NJ:
                    emit_s5(j - 8)

        # ---------------- Phase C: GEMM2 + bias2 ----------------
        with contextlib.ExitStack() as ctx:
            w2p = ctx.enter_context(tc.tile_pool(name="w2p", bufs=2))
            b2p = ctx.enter_context(tc.tile_pool(name="b2p", bufs=2))
            op_ = ctx.enter_context(tc.tile_pool(name="op", bufs=4))
            pc = ctx.enter_context(tc.tile_pool(name="pc", bufs=8, space="PSUM"))
            for ji in range(JT):
                b2t = b2p.tile([P, J2], F32)
                nc.sync.dma_start(
                    out=b2t[:], in_=bass.AP(b2_in, J2 * ji, [[0, P], [1, J2]]))
                pss = [pc.tile([P, J2], F32, name="pss", tag="pss")
                       for _ in range(MT)]
                for kc in range(NKC):
                    w2c = w2p.tile([P, KC, J2], F16)
                    if ji == 0 and kc == 0:
                        # split the very first chunk so GEMM2 starts on the
                        # first slice instead of waiting for the whole 4MB
                        for c4 in range(8):
                            nc.sync.dma_start(
                                out=w2c[:, 2 * c4:2 * (c4 + 1), :],
                                in_=w2_in[:, 2 * c4:2 * (c4 + 1), 0:J2])
                    else:
                        nc.sync.dma_start(
                            out=w2c[:],
                            in_=w2_in[:, KC * kc:KC * (kc + 1),
                                      J2 * ji:J2 * (ji + 1)])
                    for mi in range(MT):
                        for kb in range(KC):
                            nc.tensor.matmul(
                                pss[mi][:],
                                hT[:, mi, KC * kc + kb, :],
                                w2c[:, kb, :],
                                start=(kc == 0 and kb == 0),
                                stop=(kc == NKC - 1 and kb == KC - 1))
                for mi in range(MT):
                    ot = op_.tile([P, J2], F32)
                    nc.vector.tensor_tensor(
                        ot[:], pss[mi][:], b2t[:], op=mybir.AluOpType.add)
                    nc.scalar.dma_start(
                        out=out[P * mi:P * (mi + 1), J2 * ji:J2 * (ji + 1)],
                        in_=ot[:])

    nc.compile()
    return nc


_NC = None
last_results = None


def _get_nc():
    global _NC
    if _NC is None:
        _NC = _build()
    return _NC


def _fq8_rows(w: np.ndarray) -> np.ndarray:
    """Reference fp8 row-blockwise fake-quant (bitwise-exact, OCP e4m3fn)."""
    K, N = w.shape
    wb = w.reshape(K // P, P, N)
    scale = (np.maximum(np.abs(wb).max(axis=1, keepdims=True), EPS)
             / np.float32(448.0)).astype(np.float32)
    q = (wb / scale).astype(ml_dtypes.float8_e4m3fn).astype(np.float32) * scale
    return q.reshape(K, N).astype(np.float32)


def _prepare_in_maps(x, kernel1, bias1, kernel2, bias2):
    x = np.ascontiguousarray(np.asarray(x, dtype=np.float32))
    k1 = np.asarray(kernel1, dtype=np.float32)
    k2 = np.asarray(kernel2, dtype=np.float32)
    b1 = np.ascontiguousarray(np.asarray(bias1, dtype=np.float32))
    b2 = np.ascontiguousarray(np.asarray(bias2, dtype=np.float32))

    # Host-side static weight fake-quant (+ packing, fp16).
    w1q = _fq8_rows(k1)
    w2q = _fq8_rows(k2)
    # pack [K, N] -> [P, K//P, N]  (partition-major)
    w1p = np.ascontiguousarray(
        w1q.reshape(KB1, P, EXPERT).transpose(1, 0, 2).astype(np.float16))
    w2p = np.ascontiguousarray(
        w2q.reshape(KB2, P, EXPERT).transpose(1, 0, 2).astype(np.float16))

    xf = x.reshape(ROWS, D_MODEL)
    in_maps = []
    for c in range(NCORES):
        xs = xf[MC * c:MC * (c + 1)]
        xp = np.ascontiguousarray(xs.reshape(MT, P, D_MODEL).transpose(1, 0, 2))
        in_maps.append({"xp": xp, "w1p": w1p, "b1": b1, "w2p": w2p, "b2": b2})
    return in_maps


def kernel(x, kernel1, bias1, kernel2, bias2):
    global last_results
    nc = _get_nc()
    in_maps = _prepare_in_maps(x, kernel1, bias1, kernel2, bias2)
    last_results = run_bass_kernel_spmd(nc, in_maps, core_ids=list(range(NCORES)))
    outs = [last_results.results[c]["out"] for c in range(NCORES)]
    full = np.concatenate(outs, axis=0).reshape(4, 2048, EXPERT)
    return full.astype(np.float32)


# revision 42
# speedup vs baseline: 1.0032x; 1.0032x over previous
"""Trainium2 Bass kernel for ExpertBranch: fp8-blockwise-fakequant FFN.

  h   = gelu_tanh(fq8(x) @ fq8_rows(kernel1) + bias1)
  out = fq8(h) @ fq8_rows(kernel2) + bias2

Sharding: data-parallel over the 8192 flattened rows of x — each of the 8
NeuronCores computes a 1024-row slice with replicated weights. No collectives.

Device pipeline per core (M=1024 rows), single fused region:
  A: x blockwise-fp8 fake-quant (halved-scale TRN-e4m3 trick, exact reference
     grid) -> fp16 dequant -> XBAR DMA-transpose into resident xT (SBUF).
  B: GEMM1 (fp16 operands, fp32 PSUM) + bias1 + exact tanh-gelu chain spread
     across DVE/Act/Pool + h fake-quant -> fp16 dequant -> XBAR DMA-transpose
     into resident hT (SBUF, no DRAM round-trip).
  C: GEMM2 (fp16 x fp16) streaming w2q once + bias2 -> out.

Weights are fake-quantized on the host (numpy, bitwise-exact OCP e4m3fn
semantics) and stored fp16 — weight quantization is static preprocessing; all
activation work (x-quant, GEMMs, gelu, h-quant) runs on device.  fp16 (not
bf16) keeps operand rounding at 2^-11, which matters because GEMM1 noise is
amplified by h-quant grid-boundary flips.
"""

import contextlib
import sys

import numpy as np

sys.path.insert(0, "/opt/trn_rl_repo")

import ml_dtypes  # noqa: E402

import concourse.bacc as bacc  # noqa: E402
import concourse.bass as bass  # noqa: E402
import concourse.mybir as mybir  # noqa: E402
import concourse.tile as tile  # noqa: E402
from concourse.bass_utils import run_bass_kernel_spmd  # noqa: E402

F32 = mybir.dt.float32
F16 = mybir.dt.float16
FP8 = mybir.dt.float8e4

P = 128          # partitions
NCORES = 8
D_MODEL = 2048
EXPERT = 8192
ROWS = 4 * 2048  # flattened x rows
MC = ROWS // NCORES   # rows per core = 1024
MT = MC // P          # m-tiles per core = 8
KB1 = D_MODEL // P    # k-blocks GEMM1 = 16
J1 = 256              # GEMM1 n-tile
NT1 = EXPERT // J1    # n-tiles GEMM1 = 32
NBJ = J1 // P         # fp8 blocks per GEMM1 n-tile = 2
KB2 = EXPERT // P     # k-blocks GEMM2 = 64
KC = 8                # k-blocks per w2 stream chunk
NKC = KB2 // KC       # chunks = 8
JT = EXPERT // 512    # j-tiles GEMM2 = 16
J2 = 512

C1 = float(np.float32(np.sqrt(2.0 / np.pi)))
GA = float(np.float32(0.044715))
C224INV = float(np.float32(1.0 / 224.0))
C448INV = float(np.float32(1.0 / 448.0))
EPS = 1e-12
HD = D_MODEL // 4     # phase-A quarter-tile width = 512
HKB = KB1 // 4        # k-blocks per quarter-tile = 4


def _build():
    nc = bacc.Bacc("TRN2", target_bir_lowering=False, debug=False)

    # Packed inputs (host-prepared layouts; see kernel() below).
    x_in = nc.dram_tensor("xp", [P, MT, D_MODEL], F32, kind="ExternalInput")
    w1_in = nc.dram_tensor("w1p", [P, KB1, EXPERT], F16, kind="ExternalInput")
    b1_in = nc.dram_tensor("b1", [EXPERT], F32, kind="ExternalInput")
    w2_in = nc.dram_tensor("w2p", [P, KB2, EXPERT], F16, kind="ExternalInput")
    b2_in = nc.dram_tensor("b2", [EXPERT], F32, kind="ExternalInput")
    out = nc.dram_tensor("out", [MC, EXPERT], F32, kind="ExternalOutput")

    with tile.TileContext(nc) as tc, contextlib.ExitStack() as top:
        resid = top.enter_context(tc.tile_pool(name="resid", bufs=1))
        # xT[p, mi, kb, m'] = xq_mi[m', kb*128+p]   (32 KiB/part)
        xT = resid.tile([P, MT, KB1, P], F16)
        # hT[p, mi, kb, m'] = hq_mi[m', kb*128+p]   (128 KiB/part)
        hT = resid.tile([P, MT, KB2, P], F16)

        # ---- Phases A+B share one pool scope so SBUF reuse deps don't
        # serialize the x pipeline against GEMM1.
        with contextlib.ExitStack() as ctx:
            # Phase B pools
            w1p = ctx.enter_context(tc.tile_pool(name="w1p", bufs=2))
            b1p = ctx.enter_context(tc.tile_pool(name="b1p", bufs=2))
            zp = ctx.enter_context(tc.tile_pool(name="zp", bufs=5))
            hp2 = ctx.enter_context(tc.tile_pool(name="hp2", bufs=5))
            scb = ctx.enter_context(tc.tile_pool(name="scb", bufs=6))
            q8b = ctx.enter_context(tc.tile_pool(name="q8b", bufs=2))
            hqp = ctx.enter_context(tc.tile_pool(name="hqp", bufs=5))
            pp = ctx.enter_context(tc.tile_pool(name="pp", bufs=8, space="PSUM"))

            # head: first two n-tiles interleaved mi-major so early GEMM1
            # work unlocks per x-unit during phase A; tail: ni-major.
            jobs = [(ni, mi) for mi in range(MT) for ni in (0, 1)]
            jobs += [(ni, mi) for ni in range(2, NT1) for mi in range(MT)]
            NJ = len(jobs)
            st = {}
            w1ts = {}
            b1ts = {}

            def load_ni(ni):
                # chunked so no single DMA holds the DMA engines long enough
                # to stall a transpose dispatch behind it
                w1t = w1p.tile([P, KB1, J1], F16)
                for c in range(4):
                    nc.sync.dma_start(
                        out=w1t[:, 4 * c:4 * (c + 1), :],
                        in_=w1_in[:, 4 * c:4 * (c + 1), J1 * ni:J1 * (ni + 1)])
                b1t = b1p.tile([P, J1], F32)
                nc.sync.dma_start(
                    out=b1t[:], in_=bass.AP(b1_in, J1 * ni, [[0, P], [1, J1]]))
                w1ts[ni] = w1t
                b1ts[ni] = b1t


            # ---------------- Phase A: x quant -> fp16 -> transpose --------
            # A pools live in their own scope, closed right after emission so
            # phase C's w2 stream buffers can reuse these bytes (their DMAs
            # then depend only on long-finished A ops -> prefetch during B).
            actx = contextlib.ExitStack()
            xa = actx.enter_context(tc.tile_pool(name="xa", bufs=3))
            sca = actx.enter_context(tc.tile_pool(name="sca", bufs=3))
            q8a = actx.enter_context(tc.tile_pool(name="q8a", bufs=2))
            xqa = actx.enter_context(tc.tile_pool(name="xqa", bufs=2))
            # Skewed stages (load / amax+quant / dequant / transpose) so the
            # cross-engine ring pipelines instead of serializing per unit.
            # No PE instructions here: GEMM1 below starts as soon as the
            # first xT slices land.
            NA = MT * 4
            ast = {}

            def a_load(k):
                mi, h = divmod(k, 4)
                xt = xa.tile([P, HD], F32)
                nc.sync.dma_start(
                    out=xt[:], in_=x_in[:, mi, HD * h:HD * (h + 1)])
                if k == 0:
                    load_ni(0)
                    load_ni(1)
                ast[k] = {"xt": xt}

            def a_scales(k):
                s = ast[k]
                xv3 = s["xt"][:].rearrange("p (kb b) -> p kb b", b=P)
                amax = sca.tile([P, HKB], F32, tag="amax")
                nc.vector.tensor_reduce(
                    amax[:], xv3, axis=mybir.AxisListType.X,
                    op=mybir.AluOpType.max, apply_absolute_value=True)
                nc.vector.tensor_scalar_max(amax[:], amax[:], EPS)
                rcp = sca.tile([P, HKB], F32, tag="rcp")
                nc.vector.reciprocal(rcp[:], amax[:])
                inv2 = sca.tile([P, HKB], F32, tag="inv2")
                nc.vector.tensor_scalar_mul(inv2[:], rcp[:], 224.0)
                s2 = sca.tile([P, HKB], F32, tag="s2")
                nc.vector.tensor_scalar_mul(s2[:], amax[:], C224INV)
                s["inv2"] = inv2
                s["s2"] = s2

            def a_quant(k):
                s = ast[k]
                q8 = q8a.tile([P, HD], FP8)
                for kb in range(HKB):
                    sl = slice(P * kb, P * (kb + 1))
                    # fp8 code: RNE(fl32(x * (224/amax))), split Act/DVE
                    if kb % 4 < 2:
                        nc.scalar.activation(
                            q8[:, sl], s["xt"][:, sl],
                            mybir.ActivationFunctionType.Copy,
                            scale=s["inv2"][:, kb:kb + 1])
                    else:
                        nc.vector.tensor_scalar(
                            q8[:, sl], s["xt"][:, sl], s["inv2"][:, kb:kb + 1],
                            None, op0=mybir.AluOpType.mult)
                s["q8"] = q8

            def a_dequant(k):
                s = ast[k]
                xq = xqa.tile([P, HD], F16)
                for kb in range(HKB):
                    sl = slice(P * kb, P * (kb + 1))
                    # dequant: fp16(code * fl(amax/224)), split Pool/Act
                    if kb % 4 < 3:
                        nc.gpsimd.tensor_scalar(
                            xq[:, sl], s["q8"][:, sl], s["s2"][:, kb:kb + 1],
                            None, op0=mybir.AluOpType.mult)
                    else:
                        nc.scalar.activation(
                            xq[:, sl], s["q8"][:, sl],
                            mybir.ActivationFunctionType.Copy,
                            scale=s["s2"][:, kb:kb + 1])
                s["xq"] = xq

            def a_transpose(k):
                mi, h = divmod(k, 4)
                s = ast.pop(k)
                nc.sync.dma_start_transpose(
                    out=xT[:, mi, HKB * h:HKB * (h + 1), :], in_=s["xq"][:])

            for k in range(NA + 4):
                if k < NA:
                    a_load(k)
                if 0 <= k - 1 < NA:
                    a_scales(k - 1)
                if 0 <= k - 2 < NA:
                    a_quant(k - 2)
                if 0 <= k - 3 < NA:
                    a_dequant(k - 3)
                if 0 <= k - 4 < NA:
                    a_transpose(k - 4)
            actx.close()

            # ------- Phase B: GEMM1 + bias + gelu + h-quant + transpose ----
            # Software-pipelined emission: stage s of tile j is emitted next
            # to stage s-1 of tile j+1 so no in-order engine queue ever
            # blocks on a same-tile cross-engine dependency.

            def emit_s0(j):
                ni, mi = jobs[j]
                tgt = 2 if j == 13 else (ni + 1 if j >= 16 and mi == 0 else -1)
                if 0 <= tgt < NT1 and tgt not in w1ts:
                    load_ni(tgt)
                ps = pp.tile([P, J1], F32)
                for kb in range(KB1):
                    nc.tensor.matmul(
                        ps[:], xT[:, mi, kb, :], w1ts[ni][:, kb, :],
                        start=(kb == 0), stop=(kb == KB1 - 1))
                z = zp.tile([P, J1], F32, tag="z")
                nc.vector.tensor_tensor(
                    z[:], ps[:], b1ts[ni][:], op=mybir.AluOpType.add)
                st[j] = {"z": z}

            def emit_s2(j):
                s = st[j]
                # h = gelu_tanh(z) via the hardware act table (matches the
                # jax tanh-approx gelu formula)
                h = hp2.tile([P, J1], F32, tag="h")
                nc.scalar.activation(
                    h[:], s["z"][:],
                    mybir.ActivationFunctionType.Gelu_apprx_tanh)
                s["h"] = h

            def emit_s3(j):
                s = st[j]
                amaxh = scb.tile([P, NBJ], F32, tag="amaxh")
                nc.vector.tensor_reduce(
                    amaxh[:], s["h"][:].rearrange("p (nb b) -> p nb b", b=P),
                    axis=mybir.AxisListType.X,
                    op=mybir.AluOpType.max, apply_absolute_value=True)
                nc.vector.tensor_scalar_max(amaxh[:], amaxh[:], EPS)
                rch = scb.tile([P, NBJ], F32, tag="rch")
                nc.vector.reciprocal(rch[:], amaxh[:])
                inv2h = scb.tile([P, NBJ], F32, tag="inv2h")
                nc.vector.tensor_scalar_mul(inv2h[:], rch[:], 224.0)
                s2h = scb.tile([P, NBJ], F32, tag="s2h")
                nc.vector.tensor_scalar_mul(s2h[:], amaxh[:], C224INV)
                s["inv2h"] = inv2h
                s["s2h"] = s2h

            def emit_s4(j):
                s = st[j]
                h8 = q8b.tile([P, J1], FP8, tag="h8")
                hq = hqp.tile([P, J1], F16, tag="hq")
                for b in range(NBJ):
                    sl = slice(P * b, P * (b + 1))
                    nc.scalar.activation(
                        h8[:, sl], s["h"][:, sl],
                        mybir.ActivationFunctionType.Copy,
                        scale=s["inv2h"][:, b:b + 1])
                    nc.vector.tensor_scalar(
                        hq[:, sl], h8[:, sl], s["s2h"][:, b:b + 1], None,
                        op0=mybir.AluOpType.mult)
                s["hq"] = hq

            def emit_s5(j):
                ni, mi = jobs[j]
                s = st.pop(j)
                eng = nc.scalar if j >= NJ - 8 else nc.sync
                eng.dma_start_transpose(
                    out=hT[:, mi, NBJ * ni:NBJ * (ni + 1), :], in_=s["hq"][:])

            for j in range(NJ + 8):
                if j < NJ:
                    emit_s0(j)
                if 0 <= j - 2 < NJ:
                    emit_s2(j - 2)
                if 0 <= j - 4 < NJ:
                    emit_s3(j - 4)
                if 0 <= j - 6 < NJ:
                    emit_s4(j - 6)
                if 0 <= j - 8 < NJ:
                    emit_s5(j - 8)

        # ---------------- Phase C: GEMM2 + bias2 ----------------
        with contextlib.ExitStack() as ctx:
            w2p = ctx.enter_context(tc.tile_pool(name="w2p", bufs=3))
            b2p = ctx.enter_context(tc.tile_pool(name="b2p", bufs=2))
            op_ = ctx.enter_context(tc.tile_pool(name="op", bufs=4))
            pc = ctx.enter_context(tc.tile_pool(name="pc", bufs=8, space="PSUM"))
            for ji in range(JT):
                b2t = b2p.tile([P, J2], F32)
                nc.sync.dma_start(
                    out=b2t[:], in_=bass.AP(b2_in, J2 * ji, [[0, P], [1, J2]]))
                pss = [pc.tile([P, J2], F32, name="pss", tag="pss")
                       for _ in range(MT)]
                for kc in range(NKC):
                    w2c = w2p.tile([P, KC, J2], F16)
                    if ji == 0 and kc == 0:
                        # split the very first chunk so GEMM2 starts on the
                        # first slice instead of waiting for the whole 4MB
                        for c4 in range(4):
                            nc.sync.dma_start(
                                out=w2c[:, 2 * c4:2 * (c4 + 1), :],
                                in_=w2_in[:, 2 * c4:2 * (c4 + 1), 0:J2])
                    else:
                        nc.sync.dma_start(
                            out=w2c[:],
                            in_=w2_in[:, KC * kc:KC * (kc + 1),
                                      J2 * ji:J2 * (ji + 1)])
                    for mi in range(MT):
                        for kb in range(KC):
                            nc.tensor.matmul(
                                pss[mi][:],
                                hT[:, mi, KC * kc + kb, :],
                                w2c[:, kb, :],
                                start=(kc == 0 and kb == 0),
                                stop=(kc == NKC - 1 and kb == KC - 1))
                for mi in range(MT):
                    ot = op_.tile([P, J2], F32)
                    nc.vector.tensor_tensor(
                        ot[:], pss[mi][:], b2t[:], op=mybir.AluOpType.add)
                    nc.scalar.dma_start(
                        out=out[P * mi:P * (mi + 1), J2 * ji:J2 * (ji + 1)],
                        in_=ot[:])

    nc.compile()
    return nc


_NC = None
last_results = None


def _get_nc():
    global _NC
    if _NC is None:
        _NC = _build()
    return _NC


def _fq8_rows(w: np.ndarray) -> np.ndarray:
    """Reference fp8 row-blockwise fake-quant (bitwise-exact, OCP e4m3fn)."""
    K, N = w.shape
    wb = w.reshape(K // P, P, N)
    scale = (np.maximum(np.abs(wb).max(axis=1, keepdims=True), EPS)
             / np.float32(448.0)).astype(np.float32)
    q = (wb / scale).astype(ml_dtypes.float8_e4m3fn).astype(np.float32) * scale
    return q.reshape(K, N).astype(np.float32)


def _prepare_in_maps(x, kernel1, bias1, kernel2, bias2):
    x = np.ascontiguousarray(np.asarray(x, dtype=np.float32))
    k1 = np.asarray(kernel1, dtype=np.float32)
    k2 = np.asarray(kernel2, dtype=np.float32)
    b1 = np.ascontiguousarray(np.asarray(bias1, dtype=np.float32))
    b2 = np.ascontiguousarray(np.asarray(bias2, dtype=np.float32))

    # Host-side static weight fake-quant (+ packing, fp16).
    w1q = _fq8_rows(k1)
    w2q = _fq8_rows(k2)
    # pack [K, N] -> [P, K//P, N]  (partition-major)
    w1p = np.ascontiguousarray(
        w1q.reshape(KB1, P, EXPERT).transpose(1, 0, 2).astype(np.float16))
    w2p = np.ascontiguousarray(
        w2q.reshape(KB2, P, EXPERT).transpose(1, 0, 2).astype(np.float16))

    xf = x.reshape(ROWS, D_MODEL)
    in_maps = []
    for c in range(NCORES):
        xs = xf[MC * c:MC * (c + 1)]
        xp = np.ascontiguousarray(xs.reshape(MT, P, D_MODEL).transpose(1, 0, 2))
        in_maps.append({"xp": xp, "w1p": w1p, "b1": b1, "w2p": w2p, "b2": b2})
    return in_maps


def kernel(x, kernel1, bias1, kernel2, bias2):
    global last_results
    nc = _get_nc()
    in_maps = _prepare_in_maps(x, kernel1, bias1, kernel2, bias2)
    last_results = run_bass_kernel_spmd(nc, in_maps, core_ids=list(range(NCORES)))
    outs = [last_results.results[c]["out"] for c in range(NCORES)]
    full = np.concatenate(outs, axis=0).reshape(4, 2048, EXPERT)
    return full.astype(np.float32)


# revision 43
# speedup vs baseline: 1.0039x; 1.0007x over previous
"""Trainium2 Bass kernel for ExpertBranch: fp8-blockwise-fakequant FFN.

  h   = gelu_tanh(fq8(x) @ fq8_rows(kernel1) + bias1)
  out = fq8(h) @ fq8_rows(kernel2) + bias2

Sharding: data-parallel over the 8192 flattened rows of x — each of the 8
NeuronCores computes a 1024-row slice with replicated weights. No collectives.

Device pipeline per core (M=1024 rows), single fused region:
  A: x blockwise-fp8 fake-quant (halved-scale TRN-e4m3 trick, exact reference
     grid) -> fp16 dequant -> XBAR DMA-transpose into resident xT (SBUF).
  B: GEMM1 (fp16 operands, fp32 PSUM) + bias1 + exact tanh-gelu chain spread
     across DVE/Act/Pool + h fake-quant -> fp16 dequant -> XBAR DMA-transpose
     into resident hT (SBUF, no DRAM round-trip).
  C: GEMM2 (fp16 x fp16) streaming w2q once + bias2 -> out.

Weights are fake-quantized on the host (numpy, bitwise-exact OCP e4m3fn
semantics) and stored fp16 — weight quantization is static preprocessing; all
activation work (x-quant, GEMMs, gelu, h-quant) runs on device.  fp16 (not
bf16) keeps operand rounding at 2^-11, which matters because GEMM1 noise is
amplified by h-quant grid-boundary flips.
"""

import contextlib
import sys

import numpy as np

sys.path.insert(0, "/opt/trn_rl_repo")

import ml_dtypes  # noqa: E402

import concourse.bacc as bacc  # noqa: E402
import concourse.bass as bass  # noqa: E402
import concourse.mybir as mybir  # noqa: E402
import concourse.tile as tile  # noqa: E402
from concourse.bass_utils import run_bass_kernel_spmd  # noqa: E402

F32 = mybir.dt.float32
F16 = mybir.dt.float16
FP8 = mybir.dt.float8e4

P = 128          # partitions
NCORES = 8
D_MODEL = 2048
EXPERT = 8192
ROWS = 4 * 2048  # flattened x rows
MC = ROWS // NCORES   # rows per core = 1024
MT = MC // P          # m-tiles per core = 8
KB1 = D_MODEL // P    # k-blocks GEMM1 = 16
J1 = 256              # GEMM1 n-tile
NT1 = EXPERT // J1    # n-tiles GEMM1 = 32
NBJ = J1 // P         # fp8 blocks per GEMM1 n-tile = 2
KB2 = EXPERT // P     # k-blocks GEMM2 = 64
KC = 8                # k-blocks per w2 stream chunk
NKC = KB2 // KC       # chunks = 8
JT = EXPERT // 512    # j-tiles GEMM2 = 16
J2 = 512

C1 = float(np.float32(np.sqrt(2.0 / np.pi)))
GA = float(np.float32(0.044715))
C224INV = float(np.float32(1.0 / 224.0))
C448INV = float(np.float32(1.0 / 448.0))
EPS = 1e-12
HD = D_MODEL // 4     # phase-A quarter-tile width = 512
HKB = KB1 // 4        # k-blocks per quarter-tile = 4


def _build():
    nc = bacc.Bacc("TRN2", target_bir_lowering=False, debug=False)

    # Packed inputs (host-prepared layouts; see kernel() below).
    x_in = nc.dram_tensor("xp", [P, MT, D_MODEL], F32, kind="ExternalInput")
    w1_in = nc.dram_tensor("w1p", [P, KB1, EXPERT], F16, kind="ExternalInput")
    b1_in = nc.dram_tensor("b1", [EXPERT], F32, kind="ExternalInput")
    w2_in = nc.dram_tensor("w2p", [P, KB2, EXPERT], F16, kind="ExternalInput")
    b2_in = nc.dram_tensor("b2", [EXPERT], F32, kind="ExternalInput")
    out = nc.dram_tensor("out", [MC, EXPERT], F32, kind="ExternalOutput")

    with tile.TileContext(nc) as tc, contextlib.ExitStack() as top:
        resid = top.enter_context(tc.tile_pool(name="resid", bufs=1))
        # xT[p, mi, kb, m'] = xq_mi[m', kb*128+p]   (32 KiB/part)
        xT = resid.tile([P, MT, KB1, P], F16)
        # hT[p, mi, kb, m'] = hq_mi[m', kb*128+p]   (128 KiB/part)
        hT = resid.tile([P, MT, KB2, P], F16)

        # ---- Phases A+B share one pool scope so SBUF reuse deps don't
        # serialize the x pipeline against GEMM1.
        with contextlib.ExitStack() as ctx:
            # Phase B pools
            w1p = ctx.enter_context(tc.tile_pool(name="w1p", bufs=2))
            b1p = ctx.enter_context(tc.tile_pool(name="b1p", bufs=2))
            gp = ctx.enter_context(tc.tile_pool(name="gp", bufs=5))
            scb = ctx.enter_context(tc.tile_pool(name="scb", bufs=6))
            q8b = ctx.enter_context(tc.tile_pool(name="q8b", bufs=2))
            hqp = ctx.enter_context(tc.tile_pool(name="hqp", bufs=5))
            pp = ctx.enter_context(tc.tile_pool(name="pp", bufs=8, space="PSUM"))

            # head: first two n-tiles interleaved mi-major so early GEMM1
            # work unlocks per x-unit during phase A; tail: ni-major.
            jobs = [(ni, mi) for mi in range(MT) for ni in (0, 1)]
            jobs += [(ni, mi) for ni in range(2, NT1) for mi in range(MT)]
            NJ = len(jobs)
            st = {}
            w1ts = {}
            b1ts = {}

            def load_ni(ni):
                # chunked so no single DMA holds the DMA engines long enough
                # to stall a transpose dispatch behind it
                w1t = w1p.tile([P, KB1, J1], F16)
                for c in range(4):
                    nc.sync.dma_start(
                        out=w1t[:, 4 * c:4 * (c + 1), :],
                        in_=w1_in[:, 4 * c:4 * (c + 1), J1 * ni:J1 * (ni + 1)])
                b1t = b1p.tile([P, J1], F32)
                nc.sync.dma_start(
                    out=b1t[:], in_=bass.AP(b1_in, J1 * ni, [[0, P], [1, J1]]))
                w1ts[ni] = w1t
                b1ts[ni] = b1t


            # ---------------- Phase A: x quant -> fp16 -> transpose --------
            # A pools live in their own scope, closed right after emission so
            # phase C's w2 stream buffers can reuse these bytes (their DMAs
            # then depend only on long-finished A ops -> prefetch during B).
            actx = contextlib.ExitStack()
            xa = actx.enter_context(tc.tile_pool(name="xa", bufs=3))
            sca = actx.enter_context(tc.tile_pool(name="sca", bufs=3))
            q8a = actx.enter_context(tc.tile_pool(name="q8a", bufs=2))
            xqa = actx.enter_context(tc.tile_pool(name="xqa", bufs=2))
            # Skewed stages (load / amax+quant / dequant / transpose) so the
            # cross-engine ring pipelines instead of serializing per unit.
            # No PE instructions here: GEMM1 below starts as soon as the
            # first xT slices land.
            NA = MT * 4
            ast = {}

            def a_load(k):
                mi, h = divmod(k, 4)
                xt = xa.tile([P, HD], F32)
                nc.sync.dma_start(
                    out=xt[:], in_=x_in[:, mi, HD * h:HD * (h + 1)])
                if k == 0:
                    load_ni(0)
                    load_ni(1)
                ast[k] = {"xt": xt}

            def a_scales(k):
                s = ast[k]
                xv3 = s["xt"][:].rearrange("p (kb b) -> p kb b", b=P)
                amax = sca.tile([P, HKB], F32, tag="amax")
                nc.vector.tensor_reduce(
                    amax[:], xv3, axis=mybir.AxisListType.X,
                    op=mybir.AluOpType.max, apply_absolute_value=True)
                nc.vector.tensor_scalar_max(amax[:], amax[:], EPS)
                rcp = sca.tile([P, HKB], F32, tag="rcp")
                nc.vector.reciprocal(rcp[:], amax[:])
                inv2 = sca.tile([P, HKB], F32, tag="inv2")
                nc.vector.tensor_scalar_mul(inv2[:], rcp[:], 224.0)
                s2 = sca.tile([P, HKB], F32, tag="s2")
                nc.vector.tensor_scalar_mul(s2[:], amax[:], C224INV)
                s["inv2"] = inv2
                s["s2"] = s2

            def a_quant(k):
                s = ast[k]
                q8 = q8a.tile([P, HD], FP8)
                for kb in range(HKB):
                    sl = slice(P * kb, P * (kb + 1))
                    # fp8 code: RNE(fl32(x * (224/amax))), split Act/DVE
                    if kb % 4 < 2:
                        nc.scalar.activation(
                            q8[:, sl], s["xt"][:, sl],
                            mybir.ActivationFunctionType.Copy,
                            scale=s["inv2"][:, kb:kb + 1])
                    else:
                        nc.vector.tensor_scalar(
                            q8[:, sl], s["xt"][:, sl], s["inv2"][:, kb:kb + 1],
                            None, op0=mybir.AluOpType.mult)
                s["q8"] = q8

            def a_dequant(k):
                s = ast[k]
                xq = xqa.tile([P, HD], F16)
                for kb in range(HKB):
                    sl = slice(P * kb, P * (kb + 1))
                    # dequant: fp16(code * fl(amax/224)), split Pool/Act
                    if kb % 4 < 3:
                        nc.gpsimd.tensor_scalar(
                            xq[:, sl], s["q8"][:, sl], s["s2"][:, kb:kb + 1],
                            None, op0=mybir.AluOpType.mult)
                    else:
                        nc.scalar.activation(
                            xq[:, sl], s["q8"][:, sl],
                            mybir.ActivationFunctionType.Copy,
                            scale=s["s2"][:, kb:kb + 1])
                s["xq"] = xq

            def a_transpose(k):
                mi, h = divmod(k, 4)
                s = ast.pop(k)
                nc.sync.dma_start_transpose(
                    out=xT[:, mi, HKB * h:HKB * (h + 1), :], in_=s["xq"][:])

            for k in range(NA + 4):
                if k < NA:
                    a_load(k)
                if 0 <= k - 1 < NA:
                    a_scales(k - 1)
                if 0 <= k - 2 < NA:
                    a_quant(k - 2)
                if 0 <= k - 3 < NA:
                    a_dequant(k - 3)
                if 0 <= k - 4 < NA:
                    a_transpose(k - 4)
            actx.close()

            # ------- Phase B: GEMM1 + bias + gelu + h-quant + transpose ----
            # Software-pipelined emission: stage s of tile j is emitted next
            # to stage s-1 of tile j+1 so no in-order engine queue ever
            # blocks on a same-tile cross-engine dependency.

            def emit_s0(j):
                ni, mi = jobs[j]
                tgt = 2 if j == 13 else (ni + 1 if j >= 16 and mi == 0 else -1)
                if 0 <= tgt < NT1 and tgt not in w1ts:
                    load_ni(tgt)
                ps = pp.tile([P, J1], F32)
                for kb in range(KB1):
                    nc.tensor.matmul(
                        ps[:], xT[:, mi, kb, :], w1ts[ni][:, kb, :],
                        start=(kb == 0), stop=(kb == KB1 - 1))
                z = gp.tile([P, J1], F32, tag="z")
                nc.vector.tensor_tensor(
                    z[:], ps[:], b1ts[ni][:], op=mybir.AluOpType.add)
                st[j] = {"z": z}

            def emit_s2(j):
                s = st[j]
                # h = gelu_tanh(z) via the hardware act table (matches the
                # jax tanh-approx gelu formula)
                h = gp.tile([P, J1], F32, tag="h")
                nc.scalar.activation(
                    h[:], s["z"][:],
                    mybir.ActivationFunctionType.Gelu_apprx_tanh)
                s["h"] = h

            def emit_s3(j):
                s = st[j]
                amaxh = scb.tile([P, NBJ], F32, tag="amaxh")
                nc.vector.tensor_reduce(
                    amaxh[:], s["h"][:].rearrange("p (nb b) -> p nb b", b=P),
                    axis=mybir.AxisListType.X,
                    op=mybir.AluOpType.max, apply_absolute_value=True)
                nc.vector.tensor_scalar_max(amaxh[:], amaxh[:], EPS)
                rch = scb.tile([P, NBJ], F32, tag="rch")
                nc.vector.reciprocal(rch[:], amaxh[:])
                inv2h = scb.tile([P, NBJ], F32, tag="inv2h")
                nc.vector.tensor_scalar_mul(inv2h[:], rch[:], 224.0)
                s2h = scb.tile([P, NBJ], F32, tag="s2h")
                nc.vector.tensor_scalar_mul(s2h[:], amaxh[:], C224INV)
                s["inv2h"] = inv2h
                s["s2h"] = s2h

            def emit_s4(j):
                s = st[j]
                h8 = q8b.tile([P, J1], FP8, tag="h8")
                hq = hqp.tile([P, J1], F16, tag="hq")
                for b in range(NBJ):
                    sl = slice(P * b, P * (b + 1))
                    nc.scalar.activation(
                        h8[:, sl], s["h"][:, sl],
                        mybir.ActivationFunctionType.Copy,
                        scale=s["inv2h"][:, b:b + 1])
                    nc.vector.tensor_scalar(
                        hq[:, sl], h8[:, sl], s["s2h"][:, b:b + 1], None,
                        op0=mybir.AluOpType.mult)
                s["hq"] = hq

            def emit_s5(j):
                ni, mi = jobs[j]
                s = st.pop(j)
                nc.sync.dma_start_transpose(
                    out=hT[:, mi, NBJ * ni:NBJ * (ni + 1), :], in_=s["hq"][:])

            for j in range(NJ + 8):
                if j < NJ:
                    emit_s0(j)
                if 0 <= j - 2 < NJ:
                    emit_s2(j - 2)
                if 0 <= j - 4 < NJ:
                    emit_s3(j - 4)
                if 0 <= j - 6 < NJ:
                    emit_s4(j - 6)
                if 0 <= j - 8 < NJ:
                    emit_s5(j - 8)

        # ---------------- Phase C: GEMM2 + bias2 ----------------
        with contextlib.ExitStack() as ctx:
            w2p = ctx.enter_context(tc.tile_pool(name="w2p", bufs=3))
            b2p = ctx.enter_context(tc.tile_pool(name="b2p", bufs=2))
            op_ = ctx.enter_context(tc.tile_pool(name="op", bufs=4))
            pc = ctx.enter_context(tc.tile_pool(name="pc", bufs=8, space="PSUM"))
            for ji in range(JT):
                b2t = b2p.tile([P, J2], F32)
                nc.sync.dma_start(
                    out=b2t[:], in_=bass.AP(b2_in, J2 * ji, [[0, P], [1, J2]]))
                pss = [pc.tile([P, J2], F32, name="pss", tag="pss")
                       for _ in range(MT)]
                for kc in range(NKC):
                    w2c = w2p.tile([P, KC, J2], F16)
                    if ji == 0 and kc == 0:
                        # split the very first chunk so GEMM2 starts on the
                        # first slice instead of waiting for the whole 2MB
                        for c4 in range(4):
                            nc.sync.dma_start(
                                out=w2c[:, 2 * c4:2 * (c4 + 1), :],
                                in_=w2_in[:, 2 * c4:2 * (c4 + 1), 0:J2])
                    else:
                        nc.sync.dma_start(
                            out=w2c[:],
                            in_=w2_in[:, KC * kc:KC * (kc + 1),
                                      J2 * ji:J2 * (ji + 1)])
                    for mi in range(MT):
                        for kb in range(KC):
                            nc.tensor.matmul(
                                pss[mi][:],
                                hT[:, mi, KC * kc + kb, :],
                                w2c[:, kb, :],
                                start=(kc == 0 and kb == 0),
                                stop=(kc == NKC - 1 and kb == KC - 1))
                for mi in range(MT):
                    ot = op_.tile([P, J2], F32)
                    nc.vector.tensor_tensor(
                        ot[:], pss[mi][:], b2t[:], op=mybir.AluOpType.add)
                    nc.scalar.dma_start(
                        out=out[P * mi:P * (mi + 1), J2 * ji:J2 * (ji + 1)],
                        in_=ot[:])

    nc.compile()
    return nc


_NC = None
last_results = None


def _get_nc():
    global _NC
    if _NC is None:
        _NC = _build()
    return _NC


def _fq8_rows(w: np.ndarray) -> np.ndarray:
    """Reference fp8 row-blockwise fake-quant (bitwise-exact, OCP e4m3fn)."""
    K, N = w.shape
    wb = w.reshape(K // P, P, N)
    scale = (np.maximum(np.abs(wb).max(axis=1, keepdims=True), EPS)
             / np.float32(448.0)).astype(np.float32)
    q = (wb / scale).astype(ml_dtypes.float8_e4m3fn).astype(np.float32) * scale
    return q.reshape(K, N).astype(np.float32)


def _prepare_in_maps(x, kernel1, bias1, kernel2, bias2):
    x = np.ascontiguousarray(np.asarray(x, dtype=np.float32))
    k1 = np.asarray(kernel1, dtype=np.float32)
    k2 = np.asarray(kernel2, dtype=np.float32)
    b1 = np.ascontiguousarray(np.asarray(bias1, dtype=np.float32))
    b2 = np.ascontiguousarray(np.asarray(bias2, dtype=np.float32))

    # Host-side static weight fake-quant (+ packing, fp16).
    w1q = _fq8_rows(k1)
    w2q = _fq8_rows(k2)
    # pack [K, N] -> [P, K//P, N]  (partition-major)
    w1p = np.ascontiguousarray(
        w1q.reshape(KB1, P, EXPERT).transpose(1, 0, 2).astype(np.float16))
    w2p = np.ascontiguousarray(
        w2q.reshape(KB2, P, EXPERT).transpose(1, 0, 2).astype(np.float16))

    xf = x.reshape(ROWS, D_MODEL)
    in_maps = []
    for c in range(NCORES):
        xs = xf[MC * c:MC * (c + 1)]
        xp = np.ascontiguousarray(xs.reshape(MT, P, D_MODEL).transpose(1, 0, 2))
        in_maps.append({"xp": xp, "w1p": w1p, "b1": b1, "w2p": w2p, "b2": b2})
    return in_maps


def kernel(x, kernel1, bias1, kernel2, bias2):
    global last_results
    nc = _get_nc()
    in_maps = _prepare_in_maps(x, kernel1, bias1, kernel2, bias2)
    last_results = run_bass_kernel_spmd(nc, in_maps, core_ids=list(range(NCORES)))
    outs = [last_results.results[c]["out"] for c in range(NCORES)]
    full = np.concatenate(outs, axis=0).reshape(4, 2048, EXPERT)
    return full.astype(np.float32)


# revision 44
# speedup vs baseline: 1.0058x; 1.0019x over previous
"""Trainium2 Bass kernel for ExpertBranch: fp8-blockwise-fakequant FFN.

  h   = gelu_tanh(fq8(x) @ fq8_rows(kernel1) + bias1)
  out = fq8(h) @ fq8_rows(kernel2) + bias2

Sharding: data-parallel over the 8192 flattened rows of x — each of the 8
NeuronCores computes a 1024-row slice with replicated weights. No collectives.

Device pipeline per core (M=1024 rows), single fused region:
  A: x blockwise-fp8 fake-quant (halved-scale TRN-e4m3 trick, exact reference
     grid) -> fp16 dequant -> XBAR DMA-transpose into resident xT (SBUF).
  B: GEMM1 (fp16 operands, fp32 PSUM) + bias1 + exact tanh-gelu chain spread
     across DVE/Act/Pool + h fake-quant -> fp16 dequant -> XBAR DMA-transpose
     into resident hT (SBUF, no DRAM round-trip).
  C: GEMM2 (fp16 x fp16) streaming w2q once + bias2 -> out.

Weights are fake-quantized on the host (numpy, bitwise-exact OCP e4m3fn
semantics) and stored fp16 — weight quantization is static preprocessing; all
activation work (x-quant, GEMMs, gelu, h-quant) runs on device.  fp16 (not
bf16) keeps operand rounding at 2^-11, which matters because GEMM1 noise is
amplified by h-quant grid-boundary flips.
"""

import contextlib
import sys

import numpy as np

sys.path.insert(0, "/opt/trn_rl_repo")

import ml_dtypes  # noqa: E402

import concourse.bacc as bacc  # noqa: E402
import concourse.bass as bass  # noqa: E402
import concourse.mybir as mybir  # noqa: E402
import concourse.tile as tile  # noqa: E402
from concourse.bass_utils import run_bass_kernel_spmd  # noqa: E402

F32 = mybir.dt.float32
F16 = mybir.dt.float16
FP8 = mybir.dt.float8e4

P = 128          # partitions
NCORES = 8
D_MODEL = 2048
EXPERT = 8192
ROWS = 4 * 2048  # flattened x rows
MC = ROWS // NCORES   # rows per core = 1024
MT = MC // P          # m-tiles per core = 8
KB1 = D_MODEL // P    # k-blocks GEMM1 = 16
J1 = 256              # GEMM1 n-tile
NT1 = EXPERT // J1    # n-tiles GEMM1 = 32
NBJ = J1 // P         # fp8 blocks per GEMM1 n-tile = 2
KB2 = EXPERT // P     # k-blocks GEMM2 = 64
KC = 8                # k-blocks per w2 stream chunk
NKC = KB2 // KC       # chunks = 8
JT = EXPERT // 512    # j-tiles GEMM2 = 16
J2 = 512

C1 = float(np.float32(np.sqrt(2.0 / np.pi)))
GA = float(np.float32(0.044715))
C224INV = float(np.float32(1.0 / 224.0))
C448INV = float(np.float32(1.0 / 448.0))
EPS = 1e-12
HD = D_MODEL // 4     # phase-A quarter-tile width = 512
HKB = KB1 // 4        # k-blocks per quarter-tile = 4


def _build():
    nc = bacc.Bacc("TRN2", target_bir_lowering=False, debug=False)

    # Packed inputs (host-prepared layouts; see kernel() below).
    x_in = nc.dram_tensor("xp", [P, MT, D_MODEL], F32, kind="ExternalInput")
    w1_in = nc.dram_tensor("w1p", [P, KB1, EXPERT], F16, kind="ExternalInput")
    b1_in = nc.dram_tensor("b1", [EXPERT], F32, kind="ExternalInput")
    w2_in = nc.dram_tensor("w2p", [P, KB2, EXPERT], F16, kind="ExternalInput")
    b2_in = nc.dram_tensor("b2", [EXPERT], F32, kind="ExternalInput")
    out = nc.dram_tensor("out", [MC, EXPERT], F32, kind="ExternalOutput")

    with tile.TileContext(nc) as tc, contextlib.ExitStack() as top:
        resid = top.enter_context(tc.tile_pool(name="resid", bufs=1))
        # xT[p, mi, kb, m'] = xq_mi[m', kb*128+p]   (32 KiB/part)
        xT = resid.tile([P, MT, KB1, P], F16)
        # hT[p, mi, kb, m'] = hq_mi[m', kb*128+p]   (128 KiB/part)
        hT = resid.tile([P, MT, KB2, P], F16)

        # ---- Phases A+B share one pool scope so SBUF reuse deps don't
        # serialize the x pipeline against GEMM1.
        with contextlib.ExitStack() as ctx:
            # Phase B pools
            w1p = ctx.enter_context(tc.tile_pool(name="w1p", bufs=2))
            b1p = ctx.enter_context(tc.tile_pool(name="b1p", bufs=2))
            gp = ctx.enter_context(tc.tile_pool(name="gp", bufs=5))
            scb = ctx.enter_context(tc.tile_pool(name="scb", bufs=6))
            q8b = ctx.enter_context(tc.tile_pool(name="q8b", bufs=2))
            hqp = ctx.enter_context(tc.tile_pool(name="hqp", bufs=5))
            pp = ctx.enter_context(tc.tile_pool(name="pp", bufs=8, space="PSUM"))

            # head: first two n-tiles interleaved mi-major so early GEMM1
            # work unlocks per x-unit during phase A; tail: ni-major.
            jobs = [(ni, mi) for mi in range(MT) for ni in (0, 1)]
            jobs += [(ni, mi) for ni in range(2, NT1) for mi in range(MT)]
            NJ = len(jobs)
            st = {}
            w1ts = {}
            b1ts = {}

            def load_ni(ni):
                # chunked so no single DMA holds the DMA engines long enough
                # to stall a transpose dispatch behind it
                w1t = w1p.tile([P, KB1, J1], F16)
                for c in range(4):
                    nc.sync.dma_start(
                        out=w1t[:, 4 * c:4 * (c + 1), :],
                        in_=w1_in[:, 4 * c:4 * (c + 1), J1 * ni:J1 * (ni + 1)])
                b1t = b1p.tile([P, J1], F32)
                nc.sync.dma_start(
                    out=b1t[:], in_=bass.AP(b1_in, J1 * ni, [[0, P], [1, J1]]))
                w1ts[ni] = w1t
                b1ts[ni] = b1t


            # ---------------- Phase A: x quant -> fp16 -> transpose --------
            # A pools live in their own scope, closed right after emission so
            # phase C's w2 stream buffers can reuse these bytes (their DMAs
            # then depend only on long-finished A ops -> prefetch during B).
            actx = contextlib.ExitStack()
            xa = actx.enter_context(tc.tile_pool(name="xa", bufs=3))
            sca = actx.enter_context(tc.tile_pool(name="sca", bufs=3))
            q8a = actx.enter_context(tc.tile_pool(name="q8a", bufs=2))
            xqa = actx.enter_context(tc.tile_pool(name="xqa", bufs=2))
            # Skewed stages (load / amax+quant / dequant / transpose) so the
            # cross-engine ring pipelines instead of serializing per unit.
            # No PE instructions here: GEMM1 below starts as soon as the
            # first xT slices land.
            NA = MT * 4
            ast = {}

            def a_load(k):
                mi, h = divmod(k, 4)
                xt = xa.tile([P, HD], F32)
                nc.sync.dma_start(
                    out=xt[:], in_=x_in[:, mi, HD * h:HD * (h + 1)])
                if k == 0:
                    load_ni(0)
                    load_ni(1)
                ast[k] = {"xt": xt}

            def a_scales(k):
                s = ast[k]
                xv3 = s["xt"][:].rearrange("p (kb b) -> p kb b", b=P)
                amax = sca.tile([P, HKB], F32, tag="amax")
                nc.vector.tensor_reduce(
                    amax[:], xv3, axis=mybir.AxisListType.X,
                    op=mybir.AluOpType.max, apply_absolute_value=True)
                nc.vector.tensor_scalar_max(amax[:], amax[:], EPS)
                rcp = sca.tile([P, HKB], F32, tag="rcp")
                nc.vector.reciprocal(rcp[:], amax[:])
                inv2 = sca.tile([P, HKB], F32, tag="inv2")
                nc.vector.tensor_scalar_mul(inv2[:], rcp[:], 224.0)
                s2 = sca.tile([P, HKB], F32, tag="s2")
                nc.vector.tensor_scalar_mul(s2[:], amax[:], C224INV)
                s["inv2"] = inv2
                s["s2"] = s2

            def a_quant(k):
                s = ast[k]
                q8 = q8a.tile([P, HD], FP8)
                for kb in range(HKB):
                    sl = slice(P * kb, P * (kb + 1))
                    # fp8 code: RNE(fl32(x * (224/amax))), split Act/DVE
                    if kb % 4 < 2:
                        nc.scalar.activation(
                            q8[:, sl], s["xt"][:, sl],
                            mybir.ActivationFunctionType.Copy,
                            scale=s["inv2"][:, kb:kb + 1])
                    else:
                        nc.vector.tensor_scalar(
                            q8[:, sl], s["xt"][:, sl], s["inv2"][:, kb:kb + 1],
                            None, op0=mybir.AluOpType.mult)
                s["q8"] = q8

            def a_dequant(k):
                s = ast[k]
                xq = xqa.tile([P, HD], F16)
                for kb in range(HKB):
                    sl = slice(P * kb, P * (kb + 1))
                    # dequant: fp16(code * fl(amax/224)), split Pool/Act
                    if kb % 4 < 3:
                        nc.gpsimd.tensor_scalar(
                            xq[:, sl], s["q8"][:, sl], s["s2"][:, kb:kb + 1],
                            None, op0=mybir.AluOpType.mult)
                    else:
                        nc.scalar.activation(
                            xq[:, sl], s["q8"][:, sl],
                            mybir.ActivationFunctionType.Copy,
                            scale=s["s2"][:, kb:kb + 1])
                s["xq"] = xq

            def a_transpose(k):
                mi, h = divmod(k, 4)
                s = ast.pop(k)
                nc.sync.dma_start_transpose(
                    out=xT[:, mi, HKB * h:HKB * (h + 1), :], in_=s["xq"][:])

            for k in range(NA + 4):
                if k < NA:
                    a_load(k)
                if 0 <= k - 1 < NA:
                    a_scales(k - 1)
                if 0 <= k - 2 < NA:
                    a_quant(k - 2)
                if 0 <= k - 3 < NA:
                    a_dequant(k - 3)
                if 0 <= k - 4 < NA:
                    a_transpose(k - 4)
            actx.close()

            # ------- Phase B: GEMM1 + bias + gelu + h-quant + transpose ----
            # Software-pipelined emission: stage s of tile j is emitted next
            # to stage s-1 of tile j+1 so no in-order engine queue ever
            # blocks on a same-tile cross-engine dependency.

            def emit_s0(j):
                ni, mi = jobs[j]
                tgt = 2 if j == 13 else (ni + 1 if j >= 16 and mi == 0 else -1)
                if 0 <= tgt < NT1 and tgt not in w1ts:
                    load_ni(tgt)
                ps = pp.tile([P, J1], F32)
                for kb in range(KB1):
                    nc.tensor.matmul(
                        ps[:], xT[:, mi, kb, :], w1ts[ni][:, kb, :],
                        start=(kb == 0), stop=(kb == KB1 - 1))
                z = gp.tile([P, J1], F32, tag="z")
                nc.vector.tensor_tensor(
                    z[:], ps[:], b1ts[ni][:], op=mybir.AluOpType.add)
                st[j] = {"z": z}

            def emit_s2(j):
                s = st[j]
                # h = gelu_tanh(z) via the hardware act table (matches the
                # jax tanh-approx gelu formula)
                h = gp.tile([P, J1], F32, tag="h")
                nc.scalar.activation(
                    h[:], s["z"][:],
                    mybir.ActivationFunctionType.Gelu_apprx_tanh)
                s["h"] = h

            def emit_s3(j):
                s = st[j]
                amaxh = scb.tile([P, NBJ], F32, tag="amaxh")
                nc.vector.tensor_reduce(
                    amaxh[:], s["h"][:].rearrange("p (nb b) -> p nb b", b=P),
                    axis=mybir.AxisListType.X,
                    op=mybir.AluOpType.max, apply_absolute_value=True)
                nc.vector.tensor_scalar_max(amaxh[:], amaxh[:], EPS)
                rch = scb.tile([P, NBJ], F32, tag="rch")
                nc.vector.reciprocal(rch[:], amaxh[:])
                inv2h = scb.tile([P, NBJ], F32, tag="inv2h")
                nc.vector.tensor_scalar_mul(inv2h[:], rch[:], 224.0)
                s2h = scb.tile([P, NBJ], F32, tag="s2h")
                nc.vector.tensor_scalar_mul(s2h[:], amaxh[:], C224INV)
                s["inv2h"] = inv2h
                s["s2h"] = s2h

            def emit_s4(j):
                s = st[j]
                h8 = q8b.tile([P, J1], FP8, tag="h8")
                hq = hqp.tile([P, J1], F16, tag="hq")
                for b in range(NBJ):
                    sl = slice(P * b, P * (b + 1))
                    nc.scalar.activation(
                        h8[:, sl], s["h"][:, sl],
                        mybir.ActivationFunctionType.Copy,
                        scale=s["inv2h"][:, b:b + 1])
                    nc.vector.tensor_scalar(
                        hq[:, sl], h8[:, sl], s["s2h"][:, b:b + 1], None,
                        op0=mybir.AluOpType.mult)
                s["hq"] = hq

            def emit_s5(j):
                ni, mi = jobs[j]
                s = st.pop(j)
                nc.sync.dma_start_transpose(
                    out=hT[:, mi, NBJ * ni:NBJ * (ni + 1), :], in_=s["hq"][:])

            for j in range(NJ + 8):
                if j < NJ:
                    emit_s0(j)
                if 0 <= j - 2 < NJ:
                    emit_s2(j - 2)
                if 0 <= j - 4 < NJ:
                    emit_s3(j - 4)
                if 0 <= j - 6 < NJ:
                    emit_s4(j - 6)
                if 0 <= j - 8 < NJ:
                    emit_s5(j - 8)

        # ---------------- Phase C: GEMM2 + bias2 ----------------
        with contextlib.ExitStack() as ctx:
            w2p = ctx.enter_context(tc.tile_pool(name="w2p", bufs=3))
            b2p = ctx.enter_context(tc.tile_pool(name="b2p", bufs=2))
            op_ = ctx.enter_context(tc.tile_pool(name="op", bufs=4))
            pc = ctx.enter_context(tc.tile_pool(name="pc", bufs=8, space="PSUM"))
            for ji in range(JT):
                b2t = b2p.tile([P, J2], F32)
                nc.sync.dma_start(
                    out=b2t[:], in_=bass.AP(b2_in, J2 * ji, [[0, P], [1, J2]]))
                pss = [pc.tile([P, J2], F32, name="pss", tag="pss")
                       for _ in range(MT)]
                for kc in range(NKC):
                    w2c = w2p.tile([P, KC, J2], F16)
                    if ji == 0 and kc < 2:
                        # first chunks via the Act hwdge queue (drains ~10us
                        # earlier than SP at the B->C seam), the very first
                        # split into slices so GEMM2 starts immediately
                        for c4 in range(4):
                            nc.scalar.dma_start(
                                out=w2c[:, 2 * c4:2 * (c4 + 1), :],
                                in_=w2_in[:, KC * kc + 2 * c4:
                                          KC * kc + 2 * (c4 + 1), 0:J2])
                    else:
                        nc.sync.dma_start(
                            out=w2c[:],
                            in_=w2_in[:, KC * kc:KC * (kc + 1),
                                      J2 * ji:J2 * (ji + 1)])
                    for mi in range(MT):
                        for kb in range(KC):
                            nc.tensor.matmul(
                                pss[mi][:],
                                hT[:, mi, KC * kc + kb, :],
                                w2c[:, kb, :],
                                start=(kc == 0 and kb == 0),
                                stop=(kc == NKC - 1 and kb == KC - 1))
                        if kc == NKC - 1:
                            # epilogue per mi right after its last matmul so
                            # the PSUM bank frees before the ji boundary
                            ot = op_.tile([P, J2], F32)
                            nc.vector.tensor_tensor(
                                ot[:], pss[mi][:], b2t[:],
                                op=mybir.AluOpType.add)
                            nc.scalar.dma_start(
                                out=out[P * mi:P * (mi + 1),
                                        J2 * ji:J2 * (ji + 1)],
                                in_=ot[:])

    nc.compile()
    return nc


_NC = None
last_results = None


def _get_nc():
    global _NC
    if _NC is None:
        _NC = _build()
    return _NC


def _fq8_rows(w: np.ndarray) -> np.ndarray:
    """Reference fp8 row-blockwise fake-quant (bitwise-exact, OCP e4m3fn)."""
    K, N = w.shape
    wb = w.reshape(K // P, P, N)
    scale = (np.maximum(np.abs(wb).max(axis=1, keepdims=True), EPS)
             / np.float32(448.0)).astype(np.float32)
    q = (wb / scale).astype(ml_dtypes.float8_e4m3fn).astype(np.float32) * scale
    return q.reshape(K, N).astype(np.float32)


def _prepare_in_maps(x, kernel1, bias1, kernel2, bias2):
    x = np.ascontiguousarray(np.asarray(x, dtype=np.float32))
    k1 = np.asarray(kernel1, dtype=np.float32)
    k2 = np.asarray(kernel2, dtype=np.float32)
    b1 = np.ascontiguousarray(np.asarray(bias1, dtype=np.float32))
    b2 = np.ascontiguousarray(np.asarray(bias2, dtype=np.float32))

    # Host-side static weight fake-quant (+ packing, fp16).
    w1q = _fq8_rows(k1)
    w2q = _fq8_rows(k2)
    # pack [K, N] -> [P, K//P, N]  (partition-major)
    w1p = np.ascontiguousarray(
        w1q.reshape(KB1, P, EXPERT).transpose(1, 0, 2).astype(np.float16))
    w2p = np.ascontiguousarray(
        w2q.reshape(KB2, P, EXPERT).transpose(1, 0, 2).astype(np.float16))

    xf = x.reshape(ROWS, D_MODEL)
    in_maps = []
    for c in range(NCORES):
        xs = xf[MC * c:MC * (c + 1)]
        xp = np.ascontiguousarray(xs.reshape(MT, P, D_MODEL).transpose(1, 0, 2))
        in_maps.append({"xp": xp, "w1p": w1p, "b1": b1, "w2p": w2p, "b2": b2})
    return in_maps


def kernel(x, kernel1, bias1, kernel2, bias2):
    global last_results
    nc = _get_nc()
    in_maps = _prepare_in_maps(x, kernel1, bias1, kernel2, bias2)
    last_results = run_bass_kernel_spmd(nc, in_maps, core_ids=list(range(NCORES)))
    outs = [last_results.results[c]["out"] for c in range(NCORES)]
    full = np.concatenate(outs, axis=0).reshape(4, 2048, EXPERT)
    return full.astype(np.float32)


# revision 45
# speedup vs baseline: 1.0059x; 1.0001x over previous
"""Trainium2 Bass kernel for ExpertBranch: fp8-blockwise-fakequant FFN.

  h   = gelu_tanh(fq8(x) @ fq8_rows(kernel1) + bias1)
  out = fq8(h) @ fq8_rows(kernel2) + bias2

Sharding: data-parallel over the 8192 flattened rows of x — each of the 8
NeuronCores computes a 1024-row slice with replicated weights. No collectives.

Device pipeline per core (M=1024 rows), single fused region:
  A: x blockwise-fp8 fake-quant (halved-scale TRN-e4m3 trick, exact reference
     grid) -> fp16 dequant -> XBAR DMA-transpose into resident xT (SBUF).
  B: GEMM1 (fp16 operands, fp32 PSUM) + bias1 + exact tanh-gelu chain spread
     across DVE/Act/Pool + h fake-quant -> fp16 dequant -> XBAR DMA-transpose
     into resident hT (SBUF, no DRAM round-trip).
  C: GEMM2 (fp16 x fp16) streaming w2q once + bias2 -> out.

Weights are fake-quantized on the host (numpy, bitwise-exact OCP e4m3fn
semantics) and stored fp16 — weight quantization is static preprocessing; all
activation work (x-quant, GEMMs, gelu, h-quant) runs on device.  fp16 (not
bf16) keeps operand rounding at 2^-11, which matters because GEMM1 noise is
amplified by h-quant grid-boundary flips.
"""

import contextlib
import sys

import numpy as np

sys.path.insert(0, "/opt/trn_rl_repo")

import ml_dtypes  # noqa: E402

import concourse.bacc as bacc  # noqa: E402
import concourse.bass as bass  # noqa: E402
import concourse.mybir as mybir  # noqa: E402
import concourse.tile as tile  # noqa: E402
from concourse.bass_utils import run_bass_kernel_spmd  # noqa: E402

F32 = mybir.dt.float32
F16 = mybir.dt.float16
FP8 = mybir.dt.float8e4

P = 128          # partitions
NCORES = 8
D_MODEL = 2048
EXPERT = 8192
ROWS = 4 * 2048  # flattened x rows
MC = ROWS // NCORES   # rows per core = 1024
MT = MC // P          # m-tiles per core = 8
KB1 = D_MODEL // P    # k-blocks GEMM1 = 16
J1 = 256              # GEMM1 n-tile
NT1 = EXPERT // J1    # n-tiles GEMM1 = 32
NBJ = J1 // P         # fp8 blocks per GEMM1 n-tile = 2
KB2 = EXPERT // P     # k-blocks GEMM2 = 64
KC = 8                # k-blocks per w2 stream chunk
NKC = KB2 // KC       # chunks = 8
JT = EXPERT // 512    # j-tiles GEMM2 = 16
J2 = 512

C1 = float(np.float32(np.sqrt(2.0 / np.pi)))
GA = float(np.float32(0.044715))
C224INV = float(np.float32(1.0 / 224.0))
C448INV = float(np.float32(1.0 / 448.0))
EPS = 1e-12
HD = D_MODEL // 4     # phase-A quarter-tile width = 512
HKB = KB1 // 4        # k-blocks per quarter-tile = 4


def _build():
    nc = bacc.Bacc("TRN2", target_bir_lowering=False, debug=False)

    # Packed inputs (host-prepared layouts; see kernel() below).
    x_in = nc.dram_tensor("xp", [P, MT, D_MODEL], F32, kind="ExternalInput")
    w1_in = nc.dram_tensor("w1p", [P, KB1, EXPERT], F16, kind="ExternalInput")
    b1_in = nc.dram_tensor("b1", [EXPERT], F32, kind="ExternalInput")
    w2_in = nc.dram_tensor("w2p", [P, KB2, EXPERT], F16, kind="ExternalInput")
    b2_in = nc.dram_tensor("b2", [EXPERT], F32, kind="ExternalInput")
    out = nc.dram_tensor("out", [MC, EXPERT], F32, kind="ExternalOutput")

    with tile.TileContext(nc) as tc, contextlib.ExitStack() as top:
        resid = top.enter_context(tc.tile_pool(name="resid", bufs=1))
        # xT[p, mi, kb, m'] = xq_mi[m', kb*128+p]   (32 KiB/part)
        xT = resid.tile([P, MT, KB1, P], F16)
        # hT[p, mi, kb, m'] = hq_mi[m', kb*128+p]   (128 KiB/part)
        hT = resid.tile([P, MT, KB2, P], F16)

        # ---- Phases A+B share one pool scope so SBUF reuse deps don't
        # serialize the x pipeline against GEMM1.
        with contextlib.ExitStack() as ctx:
            # Phase B pools
            w1p = ctx.enter_context(tc.tile_pool(name="w1p", bufs=2))
            b1p = ctx.enter_context(tc.tile_pool(name="b1p", bufs=2))
            gp = ctx.enter_context(tc.tile_pool(name="gp", bufs=5))
            scb = ctx.enter_context(tc.tile_pool(name="scb", bufs=6))
            q8b = ctx.enter_context(tc.tile_pool(name="q8b", bufs=2))
            hqp = ctx.enter_context(tc.tile_pool(name="hqp", bufs=5))
            pp = ctx.enter_context(tc.tile_pool(name="pp", bufs=8, space="PSUM"))

            # head: first two n-tiles interleaved mi-major so early GEMM1
            # work unlocks per x-unit during phase A; tail: ni-major.
            jobs = [(ni, mi) for mi in range(MT) for ni in (0, 1)]
            jobs += [(ni, mi) for ni in range(2, NT1) for mi in range(MT)]
            NJ = len(jobs)
            st = {}
            w1ts = {}
            b1ts = {}

            def load_ni(ni):
                # chunked so no single DMA holds the DMA engines long enough
                # to stall a transpose dispatch behind it
                w1t = w1p.tile([P, KB1, J1], F16)
                for c in range(4):
                    nc.sync.dma_start(
                        out=w1t[:, 4 * c:4 * (c + 1), :],
                        in_=w1_in[:, 4 * c:4 * (c + 1), J1 * ni:J1 * (ni + 1)])
                b1t = b1p.tile([P, J1], F32)
                nc.sync.dma_start(
                    out=b1t[:], in_=bass.AP(b1_in, J1 * ni, [[0, P], [1, J1]]))
                w1ts[ni] = w1t
                b1ts[ni] = b1t


            # ---------------- Phase A: x quant -> fp16 -> transpose --------
            # A pools live in their own scope, closed right after emission so
            # phase C's w2 stream buffers can reuse these bytes (their DMAs
            # then depend only on long-finished A ops -> prefetch during B).
            actx = contextlib.ExitStack()
            xa = actx.enter_context(tc.tile_pool(name="xa", bufs=3))
            sca = actx.enter_context(tc.tile_pool(name="sca", bufs=3))
            q8a = actx.enter_context(tc.tile_pool(name="q8a", bufs=2))
            xqa = actx.enter_context(tc.tile_pool(name="xqa", bufs=2))
            # Skewed stages (load / amax+quant / dequant / transpose) so the
            # cross-engine ring pipelines instead of serializing per unit.
            # No PE instructions here: GEMM1 below starts as soon as the
            # first xT slices land.
            NA = MT * 4
            ast = {}

            def a_load(k):
                mi, h = divmod(k, 4)
                xt = xa.tile([P, HD], F32)
                nc.sync.dma_start(
                    out=xt[:], in_=x_in[:, mi, HD * h:HD * (h + 1)])
                if k == 0:
                    load_ni(0)
                    load_ni(1)
                ast[k] = {"xt": xt}

            def a_scales(k):
                s = ast[k]
                xv3 = s["xt"][:].rearrange("p (kb b) -> p kb b", b=P)
                amax = sca.tile([P, HKB], F32, tag="amax")
                nc.vector.tensor_reduce(
                    amax[:], xv3, axis=mybir.AxisListType.X,
                    op=mybir.AluOpType.max, apply_absolute_value=True)
                nc.vector.tensor_scalar_max(amax[:], amax[:], EPS)
                rcp = sca.tile([P, HKB], F32, tag="rcp")
                nc.vector.reciprocal(rcp[:], amax[:])
                inv2 = sca.tile([P, HKB], F32, tag="inv2")
                nc.vector.tensor_scalar_mul(inv2[:], rcp[:], 224.0)
                s2 = sca.tile([P, HKB], F32, tag="s2")
                nc.vector.tensor_scalar_mul(s2[:], amax[:], C224INV)
                s["inv2"] = inv2
                s["s2"] = s2

            def a_quant(k):
                s = ast[k]
                q8 = q8a.tile([P, HD], FP8)
                for kb in range(HKB):
                    sl = slice(P * kb, P * (kb + 1))
                    # fp8 code: RNE(fl32(x * (224/amax))), split Act/DVE
                    if kb % 4 < 2:
                        nc.scalar.activation(
                            q8[:, sl], s["xt"][:, sl],
                            mybir.ActivationFunctionType.Copy,
                            scale=s["inv2"][:, kb:kb + 1])
                    else:
                        nc.vector.tensor_scalar(
                            q8[:, sl], s["xt"][:, sl], s["inv2"][:, kb:kb + 1],
                            None, op0=mybir.AluOpType.mult)
                s["q8"] = q8

            def a_dequant(k):
                s = ast[k]
                xq = xqa.tile([P, HD], F16)
                for kb in range(HKB):
                    sl = slice(P * kb, P * (kb + 1))
                    # dequant: fp16(code * fl(amax/224)), split Pool/Act
                    if kb % 4 < 3:
                        nc.gpsimd.tensor_scalar(
                            xq[:, sl], s["q8"][:, sl], s["s2"][:, kb:kb + 1],
                            None, op0=mybir.AluOpType.mult)
                    else:
                        nc.scalar.activation(
                            xq[:, sl], s["q8"][:, sl],
                            mybir.ActivationFunctionType.Copy,
                            scale=s["s2"][:, kb:kb + 1])
                s["xq"] = xq

            def a_transpose(k):
                mi, h = divmod(k, 4)
                s = ast.pop(k)
                nc.sync.dma_start_transpose(
                    out=xT[:, mi, HKB * h:HKB * (h + 1), :], in_=s["xq"][:])

            for k in range(NA + 4):
                if k < NA:
                    a_load(k)
                if 0 <= k - 1 < NA:
                    a_scales(k - 1)
                if 0 <= k - 2 < NA:
                    a_quant(k - 2)
                if 0 <= k - 3 < NA:
                    a_dequant(k - 3)
                if 0 <= k - 4 < NA:
                    a_transpose(k - 4)
            actx.close()

            # ------- Phase B: GEMM1 + bias + gelu + h-quant + transpose ----
            # Software-pipelined emission: stage s of tile j is emitted next
            # to stage s-1 of tile j+1 so no in-order engine queue ever
            # blocks on a same-tile cross-engine dependency.

            def emit_s0(j):
                ni, mi = jobs[j]
                tgt = 2 if j == 13 else (ni + 1 if j >= 16 and mi == 0 else -1)
                if 0 <= tgt < NT1 and tgt not in w1ts:
                    load_ni(tgt)
                ps = pp.tile([P, J1], F32)
                for kb in range(KB1):
                    nc.tensor.matmul(
                        ps[:], xT[:, mi, kb, :], w1ts[ni][:, kb, :],
                        start=(kb == 0), stop=(kb == KB1 - 1))
                z = gp.tile([P, J1], F32, tag="z")
                nc.vector.tensor_tensor(
                    z[:], ps[:], b1ts[ni][:], op=mybir.AluOpType.add)
                st[j] = {"z": z}

            def emit_s2(j):
                s = st[j]
                # h = gelu_tanh(z) via the hardware act table (matches the
                # jax tanh-approx gelu formula)
                h = gp.tile([P, J1], F32, tag="h")
                nc.scalar.activation(
                    h[:], s["z"][:],
                    mybir.ActivationFunctionType.Gelu_apprx_tanh)
                s["h"] = h

            def emit_s3(j):
                s = st[j]
                amaxh = scb.tile([P, NBJ], F32, tag="amaxh")
                nc.vector.tensor_reduce(
                    amaxh[:], s["h"][:].rearrange("p (nb b) -> p nb b", b=P),
                    axis=mybir.AxisListType.X,
                    op=mybir.AluOpType.max, apply_absolute_value=True)
                nc.vector.tensor_scalar_max(amaxh[:], amaxh[:], EPS)
                rch = scb.tile([P, NBJ], F32, tag="rch")
                nc.vector.reciprocal(rch[:], amaxh[:])
                inv2h = scb.tile([P, NBJ], F32, tag="inv2h")
                nc.vector.tensor_scalar_mul(inv2h[:], rch[:], 224.0)
                s2h = scb.tile([P, NBJ], F32, tag="s2h")
                nc.vector.tensor_scalar_mul(s2h[:], amaxh[:], C224INV)
                s["inv2h"] = inv2h
                s["s2h"] = s2h

            def emit_s4(j):
                s = st[j]
                h8 = q8b.tile([P, J1], FP8, tag="h8")
                hq = hqp.tile([P, J1], F16, tag="hq")
                for b in range(NBJ):
                    sl = slice(P * b, P * (b + 1))
                    nc.scalar.activation(
                        h8[:, sl], s["h"][:, sl],
                        mybir.ActivationFunctionType.Copy,
                        scale=s["inv2h"][:, b:b + 1])
                    nc.vector.tensor_scalar(
                        hq[:, sl], h8[:, sl], s["s2h"][:, b:b + 1], None,
                        op0=mybir.AluOpType.mult)
                s["hq"] = hq

            def emit_s5(j):
                ni, mi = jobs[j]
                s = st.pop(j)
                nc.sync.dma_start_transpose(
                    out=hT[:, mi, NBJ * ni:NBJ * (ni + 1), :], in_=s["hq"][:])

            for j in range(NJ + 8):
                # older stages first so their engine-queue ops (dequant ->
                # transpose gate) run ahead of the new tile's work
                if 0 <= j - 6 < NJ:
                    emit_s4(j - 6)
                if 0 <= j - 8 < NJ:
                    emit_s5(j - 8)
                if 0 <= j - 4 < NJ:
                    emit_s3(j - 4)
                if j < NJ:
                    emit_s0(j)
                if 0 <= j - 2 < NJ:
                    emit_s2(j - 2)

        # ---------------- Phase C: GEMM2 + bias2 ----------------
        with contextlib.ExitStack() as ctx:
            w2p = ctx.enter_context(tc.tile_pool(name="w2p", bufs=3))
            b2p = ctx.enter_context(tc.tile_pool(name="b2p", bufs=2))
            op_ = ctx.enter_context(tc.tile_pool(name="op", bufs=4))
            pc = ctx.enter_context(tc.tile_pool(name="pc", bufs=8, space="PSUM"))
            for ji in range(JT):
                b2t = b2p.tile([P, J2], F32)
                nc.sync.dma_start(
                    out=b2t[:], in_=bass.AP(b2_in, J2 * ji, [[0, P], [1, J2]]))
                pss = [pc.tile([P, J2], F32, name="pss", tag="pss")
                       for _ in range(MT)]
                for kc in range(NKC):
                    w2c = w2p.tile([P, KC, J2], F16)
                    if ji == 0 and kc < 2:
                        # first chunks via the Act hwdge queue (drains ~10us
                        # earlier than SP at the B->C seam), the very first
                        # split into slices so GEMM2 starts immediately
                        for c4 in range(4):
                            nc.scalar.dma_start(
                                out=w2c[:, 2 * c4:2 * (c4 + 1), :],
                                in_=w2_in[:, KC * kc + 2 * c4:
                                          KC * kc + 2 * (c4 + 1), 0:J2])
                    else:
                        nc.sync.dma_start(
                            out=w2c[:],
                            in_=w2_in[:, KC * kc:KC * (kc + 1),
                                      J2 * ji:J2 * (ji + 1)])
                    for mi in range(MT):
                        for kb in range(KC):
                            nc.tensor.matmul(
                                pss[mi][:],
                                hT[:, mi, KC * kc + kb, :],
                                w2c[:, kb, :],
                                start=(kc == 0 and kb == 0),
                                stop=(kc == NKC - 1 and kb == KC - 1))
                        if kc == NKC - 1:
                            # epilogue per mi right after its last matmul so
                            # the PSUM bank frees before the ji boundary
                            ot = op_.tile([P, J2], F32)
                            nc.vector.tensor_tensor(
                                ot[:], pss[mi][:], b2t[:],
                                op=mybir.AluOpType.add)
                            nc.scalar.dma_start(
                                out=out[P * mi:P * (mi + 1),
                                        J2 * ji:J2 * (ji + 1)],
                                in_=ot[:])

    nc.compile()
    return nc


_NC = None
last_results = None


def _get_nc():
    global _NC
    if _NC is None:
        _NC = _build()
    return _NC


def _fq8_rows(w: np.ndarray) -> np.ndarray:
    """Reference fp8 row-blockwise fake-quant (bitwise-exact, OCP e4m3fn)."""
    K, N = w.shape
    wb = w.reshape(K // P, P, N)
    scale = (np.maximum(np.abs(wb).max(axis=1, keepdims=True), EPS)
             / np.float32(448.0)).astype(np.float32)
    q = (wb / scale).astype(ml_dtypes.float8_e4m3fn).astype(np.float32) * scale
    return q.reshape(K, N).astype(np.float32)


def _prepare_in_maps(x, kernel1, bias1, kernel2, bias2):
    x = np.ascontiguousarray(np.asarray(x, dtype=np.float32))
    k1 = np.asarray(kernel1, dtype=np.float32)
    k2 = np.asarray(kernel2, dtype=np.float32)
    b1 = np.ascontiguousarray(np.asarray(bias1, dtype=np.float32))
    b2 = np.ascontiguousarray(np.asarray(bias2, dtype=np.float32))

    # Host-side static weight fake-quant (+ packing, fp16).
    w1q = _fq8_rows(k1)
    w2q = _fq8_rows(k2)
    # pack [K, N] -> [P, K//P, N]  (partition-major)
    w1p = np.ascontiguousarray(
        w1q.reshape(KB1, P, EXPERT).transpose(1, 0, 2).astype(np.float16))
    w2p = np.ascontiguousarray(
        w2q.reshape(KB2, P, EXPERT).transpose(1, 0, 2).astype(np.float16))

    xf = x.reshape(ROWS, D_MODEL)
    in_maps = []
    for c in range(NCORES):
        xs = xf[MC * c:MC * (c + 1)]
        xp = np.ascontiguousarray(xs.reshape(MT, P, D_MODEL).transpose(1, 0, 2))
        in_maps.append({"xp": xp, "w1p": w1p, "b1": b1, "w2p": w2p, "b2": b2})
    return in_maps


def kernel(x, kernel1, bias1, kernel2, bias2):
    global last_results
    nc = _get_nc()
    in_maps = _prepare_in_maps(x, kernel1, bias1, kernel2, bias2)
    last_results = run_bass_kernel_spmd(nc, in_maps, core_ids=list(range(NCORES)))
    outs = [last_results.results[c]["out"] for c in range(NCORES)]
    full = np.concatenate(outs, axis=0).reshape(4, 2048, EXPERT)
    return full.astype(np.float32)
